# revision 1
# baseline (speedup 1.0000x reference)
"""Trainium2 Bass kernel for nn_ActionLossDelta (action-conditioned loss with
nearest-rotation projection), data-parallel across 8 NeuronCores.

Per-sample math (see builder): branch 0/1 are scalar squared errors; branches
2/3 need R = nearest-rotation(A) for two 3x3 matrices A read from pred. The
per-sample 3x3 SVD of the reference is replaced by a closed-form spectral
method: with C = A^T A, R = A*(aI + bC + cC^2) + rank-1 det<0 correction via
the adjugate of (C - l3 I); loss uses ||R-G||^2 = 6 - 2<R,G> for rotations.

Layout: feature-major planes [128 partitions x free] with the two branches
packed side by side in the free dimension ([rot | tune] -> FW = 1024), so one
instruction stream covers both 3x3 problems. Narrow (shared) stages use the
first FN columns of a wide plane.
"""
import numpy as np

B_TOTAL = 524288
N_CORES = 8
BS = B_TOTAL // N_CORES      # 65536 samples per core
P = 128
FN = BS // P                 # 512  narrow free dim (one branch / shared stage)
FW = 2 * FN                  # 1024 wide free dim (both branches packed)

TINY = 1e-30
SIG_FLOOR = 2e-3             # sigma2/3 floor as fraction of sigma1
USE_TOUCH = False            # Bacc.compile() legalizes sync waits
USE_GP = False               # GpSimd offload measured 50% SLOWER on HW; keep DVE
USE_BF16 = True              # bf16 for the H = A^T G / <M,H> blocks

_CACHE = {}


def _build_nc(reps=1):
    import concourse.bass as bass
    import concourse.tile as tile
    import concourse.mybir as mybir
    from concourse import bacc
    from contextlib import ExitStack

    dt = mybir.dt.float32
    dtb = mybir.dt.bfloat16
    AF = mybir.ActivationFunctionType
    OP = mybir.AluOpType

    nc = bacc.Bacc(None, target_bir_lowering=False, debug=False)
    predT = nc.declare_dram_parameter("predT", [20, BS], dt, isOutput=False)
    tgtT = nc.declare_dram_parameter("tgtT", [4, BS], dt, isOutput=False)
    poseT = nc.declare_dram_parameter("poseT", [3, BS], dt, isOutput=False)
    out_d = nc.declare_dram_parameter("out", [P, 1], dt, isOutput=True)

    import bass_rust as _br

    with ExitStack() as ctx:
        NBUFS = 29 if USE_BF16 else 38
        ACT_BUFS = 8 if USE_BF16 else 13
        BH_BUFS = 28 if USE_BF16 else 1
        tc = ctx.enter_context(tile.TileContext(nc))
        pool = ctx.enter_context(tc.tile_pool(name="w", bufs=NBUFS))
        apool = ctx.enter_context(tc.tile_pool(name="acto", bufs=ACT_BUFS))
        bpool = ctx.enter_context(tc.tile_pool(name="bh", bufs=BH_BUFS))
        tpool = ctx.enter_context(tc.tile_pool(name="tch", bufs=1))

        # ---- plane allocator: SSA-style, one shared tag; Tile rotates
        # through NBUFS slots and emits releases after each value's last use
        state = {"n": 0, "live": 0, "peak": 0, "alive": 0, "apeak": 0,
                 "bfree": [], "blive": 0, "bpeak": 0}
        act_names = set()
        b_names = set()

        def alloc():
            state["n"] += 1
            state["live"] += 1
            state["peak"] = max(state["peak"], state["live"])
            return pool.tile([P, FW], dt, name=f"w{state['n']}", tag="w")

        def alloc_act():
            # dedicated slots for ACT outputs: their slots only ever see
            # ACT writes + DVE reads, so WAR+input deps collapse onto the
            # DVE semaphore and stay within ACT's 2-sync-wait ISA limit
            state["n"] += 1
            t = apool.tile([P, FW], dt, name=f"a{state['n']}", tag="acto")
            act_names.add(t.tensor.name)
            state["alive"] += 1
            state["apeak"] = max(state["apeak"], state["alive"])
            return t

        def alloc_b():
            # fresh tile per value (like the w/acto pools): object-level
            # reuse created cross-repetition WAR chains that deadlocked the
            # scheduler on reps>1 builds; tag rotation reuses slots anyway
            state["blive"] += 1
            state["bpeak"] = max(state["bpeak"], state["blive"])
            state["n"] += 1
            t = bpool.tile([P, FW], dtb, name=f"b{state['n']}", tag="bh")
            b_names.add(t.tensor.name)
            return t

        def free(*ts_):
            for t_ in ts_:
                if t_.tensor.name in b_names:
                    state["bfree"].append(t_)
                    state["blive"] -= 1
                elif t_.tensor.name in act_names:
                    state["alive"] -= 1
                else:
                    state["live"] -= 1

        V = nc.vector
        S = nc.scalar
        GP = nc.gpsimd
        from contextlib import contextmanager
        cur = {"eng": V}

        @contextmanager
        def on(eng):
            prev = cur["eng"]
            cur["eng"] = eng if USE_GP else V
            try:
                yield
            finally:
                cur["eng"] = prev

        # ---- cross-engine wait absorbers -------------------------------
        # Many ISA structs hold only 1-2 sync-wait slots, but Tile freely
        # assigns more. A tiny [P,1] DVE copy ("touch") right after each
        # DMA half / ACT output carries that single cross-engine wait; all
        # real readers get a no-sync ordering edge on the touch so the
        # engine's vector clock covers their cross-engine dep and Tile
        # emits no extra wait for them.
        tstate = {"col": 0, "tile": None, "map": {}}

        def _touch(src_t, col_off):
            if not USE_TOUCH:
                return None
            if tstate["col"] % 128 == 0:
                tstate["n_t"] = tstate.get("n_t", 0) + 1
                tstate["tile"] = tpool.tile([P, 128], dt,
                                            name=f"tch{tstate['n_t']}",
                                            tag="tch")
            col = tstate["col"] % 128
            tstate["col"] += 1
            bi = V.tensor_copy(out=tstate["tile"][:, col:col + 1],
                               in_=src_t[:, col_off:col_off + 1])
            tstate["map"].setdefault(src_t.tensor.name, []).append(bi)
            return bi

        def _dep_on_touches(bi, *tiles_):
            if not USE_TOUCH:
                return
            for t_ in tiles_:
                for ti in tstate["map"].get(t_.tensor.name, ()):
                    _br.add_dep_helper(bi.ins, ti.ins, sync=False,
                                       reason="wait-absorber")

        def vw(x, wide):
            return x[:] if wide else x[:, 0:FN]

        def tt(a, b, op, out=None, wide=True):
            o = out if out is not None else alloc()
            bi = cur["eng"].tensor_tensor(out=vw(o, wide), in0=vw(a, wide),
                                          in1=vw(b, wide), op=op)
            _dep_on_touches(bi, a, b)
            return o

        def mul(a, b, out=None, wide=True):
            return tt(a, b, OP.mult, out, wide)

        def add(a, b, out=None, wide=True):
            return tt(a, b, OP.add, out, wide)

        def sub(a, b, out=None, wide=True):
            return tt(a, b, OP.subtract, out, wide)

        def ts(a, s1, op0, s2=None, op1=None, out=None, wide=True):
            o = out if out is not None else alloc()
            if s2 is None:
                bi = cur["eng"].tensor_scalar(vw(o, wide), vw(a, wide),
                                              float(s1), None, op0)
            else:
                bi = cur["eng"].tensor_scalar(vw(o, wide), vw(a, wide),
                                              float(s1), float(s2), op0, op1)
            _dep_on_touches(bi, a)
            return o

        def stt(a, s, b, op0, op1, out=None, wide=True):
            # (a op0 s) op1 b
            o = out if out is not None else alloc()
            bi = cur["eng"].scalar_tensor_tensor(vw(o, wide), vw(a, wide),
                                                 float(s), vw(b, wide),
                                                 op0, op1)
            _dep_on_touches(bi, a, b)
            return o

        def act(a, func, scale=1.0, out=None, wide=True):
            o = out if out is not None else alloc_act()
            bi = S.activation(vw(o, wide), vw(a, wide), func, bias=0.0,
                              scale=scale)
            _dep_on_touches(bi, a)
            _touch(o, 0)
            return o

        def sq(a, out=None, wide=True):
            return act(a, AF.Square, out=out, wide=wide)

        def recip(a, out=None, wide=True):
            o = out if out is not None else alloc()
            bi = V.reciprocal_approx_fast(vw(o, wide), vw(a, wide))
            _dep_on_touches(bi, a)
            return o

        def _once():
            # ================= DMA in =================
            predR = predT.ap().rearrange("c (p f) -> c p f", p=P)
            tgtR = tgtT.ap().rearrange("c (p f) -> c p f", p=P)
            poseR = poseT.ap().rearrange("c (p f) -> c p f", p=P)

            # issue the small compute-gating DMAs (tgt/pose feed the G
            # chain; p01 feeds base01) BEFORE the 18 large pred halves so
            # round-robin queue assignment never parks them behind a 256KB
            # transfer — shortens the serial head of a single execution
            tg = []
            for r in range(4):
                t = alloc()
                nc.sync.dma_start(out=t[:, 0:FN], in_=tgtR[r])
                _touch(t, 0)
                tg.append(t)
            ps = []
            for r in range(3):
                t = alloc()
                nc.sync.dma_start(out=t[:, 0:FN], in_=poseR[r])
                _touch(t, 0)
                ps.append(t)
            p01 = []
            for r in range(2):
                t = alloc()
                nc.sync.dma_start(out=t[:, 0:FN], in_=predR[r])
                _touch(t, 0)
                p01.append(t)
            A = []           # 9 wide planes: [rot col 2+j | tune col 11+j]
            for j in range(9):
                t = alloc()
                nc.sync.dma_start(out=t[:, 0:FN], in_=predR[2 + j])
                nc.sync.dma_start(out=t[:, FN:FW], in_=predR[11 + j])
                _touch(t, 0)
                _touch(t, FN)
                A.append(t)

            # ================= G = delta rotation (narrow) =================
            def normalize3(v3):
                n2a = sq(v3[0], wide=False)
                t_ = sq(v3[1], wide=False)
                n2 = add(n2a, t_, wide=False)
                sq(v3[2], out=t_, wide=False)
                add(n2, t_, out=n2, wide=False)
                free(n2a)
                nrm = act(n2, AF.Sqrt, wide=False)
                ri = recip(nrm, wide=False)
                o = [mul(v3[k], ri, wide=False) for k in range(3)]
                free(t_, n2, nrm, ri)
                return o

            nu = normalize3(ps)
            nv = normalize3([tg[1], tg[2], tg[3]])
            free(*ps)

            # dot & clip
            dsum = mul(nu[0], nv[0], wide=False)
            d1 = mul(nu[1], nv[1], wide=False)
            add(dsum, d1, out=dsum, wide=False)
            mul(nu[2], nv[2], out=d1, wide=False)
            add(dsum, d1, out=dsum, wide=False)
            cc = ts(dsum, 1.0, OP.min, -1.0, OP.max, wide=False)
            free(dsum, d1)

            # w = nu x nv
            def crossk(a1, b1, a2, b2):
                t1 = mul(a1, b1, wide=False)
                t2 = mul(a2, b2, wide=False)
                sub(t1, t2, out=t1, wide=False)
                free(t2)
                return t1

            w0 = crossk(nu[1], nv[2], nu[2], nv[1])
            w1 = crossk(nu[2], nv[0], nu[0], nv[2])
            w2 = crossk(nu[0], nv[1], nu[1], nv[0])
            free(*nu, *nv)
            onepc = ts(cc, 1.0, OP.add, TINY, OP.max, wide=False)
            invc = recip(onepc, wide=False)
            free(onepc)

            Gn = [None] * 9  # row-major G narrow

            def gdiag(wi):
                t_ = sq(wi, wide=False)
                mul(t_, invc, out=t_, wide=False)
                add(t_, cc, out=t_, wide=False)
                return t_

            def goff(wi, wj, wk, plus):
                t_ = mul(wi, wj, wide=False)
                mul(t_, invc, out=t_, wide=False)
                tt(t_, wk, OP.add if plus else OP.subtract, out=t_, wide=False)
                return t_

            Gn[0] = gdiag(w0)
            Gn[4] = gdiag(w1)
            Gn[8] = gdiag(w2)
            Gn[1] = goff(w0, w1, w2, False)   # G01 = w0w1*inv - w2
            Gn[3] = goff(w0, w1, w2, True)    # G10 = w0w1*inv + w2
            Gn[2] = goff(w0, w2, w1, True)    # G02 = w0w2*inv + w1
            Gn[6] = goff(w0, w2, w1, False)   # G20 = w0w2*inv - w1
            Gn[5] = goff(w1, w2, w0, False)   # G12 = w1w2*inv - w0
            Gn[7] = goff(w1, w2, w0, True)    # G21 = w1w2*inv + w0
            free(w0, w1, w2, invc, cc)

            # widen G via ACT copies (both halves identical)
            GW = []
            for j in range(9):
                t = alloc_b() if USE_BF16 else alloc_act()
                S.activation(t[:, 0:FN], Gn[j][:, 0:FN], AF.Copy)
                S.activation(t[:, FN:FW], Gn[j][:, 0:FN], AF.Copy)
                GW.append(t)
            free(*Gn)

            # ============ base01 (close/trans) + masks ============
            t0, t1 = tg[0], tg[1]
            base = sub(p01[0], t1, wide=False)
            sq(base, out=base, wide=False)
            m0 = ts(t0, 0.0, OP.is_equal, wide=False)
            mul(base, m0, out=base, wide=False)
            trans = sub(p01[1], t1, wide=False)
            sq(trans, out=trans, wide=False)
            ts(t0, 1.0, OP.is_equal, out=m0, wide=False)
            mul(trans, m0, out=trans, wide=False)
            add(base, trans, out=base, wide=False)
            free(trans, m0, *p01)
            mw = alloc()
            bm1 = V.tensor_scalar(mw[:, 0:FN], t0[:, 0:FN], 2.0, None, OP.is_equal)
            bm2 = V.tensor_scalar(mw[:, FN:FW], t0[:, 0:FN], 3.0, None, OP.is_equal)
            _dep_on_touches(bm1, t0)
            _dep_on_touches(bm2, t0)
            free(*tg)

            # ================= branch pair (wide) =================
            # H = A^T G in bf16 (2x DVE mode); A converted via idle ACT
            if USE_BF16:
                Ab = []
                for j in range(9):
                    t = alloc_b()
                    S.activation(t[:], A[j][:], AF.Copy)
                    Ab.append(t)
            else:
                Ab = A
            H = []
            ha = alloc_b if USE_BF16 else alloc
            for i in range(3):
                for j in range(3):
                    x = mul(Ab[i], GW[j], out=ha())
                    y = mul(Ab[3 + i], GW[3 + j], out=ha())
                    add(x, y, out=x)
                    mul(Ab[6 + i], GW[6 + j], out=y)
                    add(x, y, out=x)
                    free(y)
                    H.append(x)
            free(*GW)
            if USE_BF16:
                free(*Ab)

            # C = A^T A (6 unique)
            def cdiag(i):
                oa = sq(A[i])
                t_ = sq(A[3 + i])
                o = add(oa, t_)
                sq(A[6 + i], out=t_)
                add(o, t_, out=o)
                free(t_, oa)
                return o

            def colprod(i, j):
                x = mul(A[i], A[j])
                y = mul(A[3 + i], A[3 + j])
                add(x, y, out=x)
                mul(A[6 + i], A[6 + j], out=y)
                add(x, y, out=x)
                free(y)
                return x

            C00 = cdiag(0)
            C11 = cdiag(1)
            C22 = cdiag(2)
            C01 = colprod(0, 1)
            C02 = colprod(0, 2)
            C12 = colprod(1, 2)

            # det(A) -> sneg mask
            with on(GP):
                x = mul(A[4], A[8])
                y = mul(A[5], A[7])
                sub(x, y, out=x)
                dta = mul(A[0], x)
                mul(A[3], A[8], out=x)
                mul(A[5], A[6], out=y)
                sub(x, y, out=x)
                mul(A[1], x, out=x)
                sub(dta, x, out=dta)
                mul(A[3], A[7], out=x)
                mul(A[4], A[6], out=y)
                sub(x, y, out=x)
                mul(A[2], x, out=x)
                add(dta, x, out=dta)
                free(x, y)
                sneg = ts(dta, 0.0, OP.is_lt)
                free(dta)
            free(*A)

            # C^2 (6 unique); squares/cross-products kept for adjugate & detB
            # bf16 copies of C for the tolerance-insensitive blocks (C2
            # offdiag, adjugate, rank-1 M update); Cardano-critical parts
            # (C2 diag -> p2, detB) stay fp32
            Cb = []
            for Cx in (C00, C01, C02, C11, C12, C22):
                t = alloc_b()
                S.activation(t[:], Cx[:], AF.Copy)
                Cb.append(t)
            Cb00, Cb01, Cb02, Cb11, Cb12, Cb22 = Cb
            s01 = sq(C01)
            s02 = sq(C02)
            s12 = sq(C12)
            cp_0212 = mul(C02, C12)
            cp_0112 = mul(C01, C12)
            cp_0102 = mul(C01, C02)
            qq = sq(C00)
            with on(GP):
                C2_00 = add(qq, s01)
                add(C2_00, s02, out=C2_00)
            sq(C11, out=qq)
            with on(GP):
                C2_11 = add(s01, qq)
                add(C2_11, s12, out=C2_11)
            sq(C22, out=qq)
            with on(GP):
                C2_22 = add(s02, s12)
                add(C2_22, qq, out=C2_22)
                free(qq)
            su = add(Cb00, Cb11, out=alloc_b())
            C2_01 = mul(Cb01, su, out=alloc_b())
            cpb_0212 = mul(Cb02, Cb12, out=alloc_b())
            add(C2_01, cpb_0212, out=C2_01)
            add(Cb00, Cb22, out=su)
            C2_02 = mul(Cb02, su, out=alloc_b())
            cpb_0112 = mul(Cb01, Cb12, out=alloc_b())
            add(C2_02, cpb_0112, out=C2_02)
            add(Cb11, Cb22, out=su)
            C2_12 = mul(Cb12, su, out=alloc_b())
            cpb_0102 = mul(Cb01, Cb02, out=alloc_b())
            add(C2_12, cpb_0102, out=C2_12)
            free(su)

            # eigenvalues via Cardano (arctan+sin form)
            q = add(C00, C11)
            add(q, C22, out=q)
            ts(q, 1.0 / 3.0, OP.mult, out=q)
            trC2 = add(C2_00, C2_11)
            add(trC2, C2_22, out=trC2)
            q2 = sq(q)
            p2 = stt(q2, -3.0, trC2, OP.mult, OP.add)
            free(trC2, q2)
            ts(p2, 0.0, OP.max, TINY, OP.add, out=p2)
            pp = act(p2, AF.Sqrt, scale=1.0 / 6.0)
            pinv = recip(pp)
            free(p2)
            b00 = sub(C00, q)
            b11 = sub(C11, q)
            b22 = sub(C22, q)
            x1 = mul(b11, b22)
            sub(x1, s12, out=x1)
            detB = mul(b00, x1)
            y1 = mul(C01, b22)
            sub(y1, cp_0212, out=y1)
            mul(C01, y1, out=y1)
            sub(detB, y1, out=detB)
            mul(b11, C02, out=x1)
            tt(cp_0112, x1, OP.subtract, out=x1)
            mul(C02, x1, out=x1)
            add(detB, x1, out=detB)
            free(x1, y1, b00, b11, b22, cp_0212, cp_0112, cp_0102)
            pinv2 = sq(pinv)
            mul(pinv2, pinv, out=pinv2)
            rr = mul(detB, pinv2)
            free(detB, pinv, pinv2)
            ts(rr, 0.5, OP.mult, 1.0, OP.min, out=rr)
            ts(rr, -1.0, OP.max, out=rr)
            omr2 = sq(rr)
            ts(omr2, -1.0, OP.mult, 1.0, OP.add, out=omr2)
            ts(omr2, TINY, OP.max, out=omr2)
            sr = act(omr2, AF.Sqrt)
            rsi = recip(sr)
            free(omr2, sr)
            targ = mul(rr, rsi)
            free(rr, rsi)
            tat = act(targ, AF.Arctan)
            free(targ)
            a1 = ts(tat, -1.0 / 3.0, OP.mult, float(2 * np.pi / 3), OP.add)
            a3 = ts(tat, -1.0 / 3.0, OP.mult, float(-2 * np.pi / 3), OP.add)
            free(tat)
            c1 = act(a1, AF.Sin)
            c3 = act(a3, AF.Sin)
            free(a1, a3)
            l1 = mul(pp, c1)
            stt(l1, 2.0, q, OP.mult, OP.add, out=l1)
            l3 = mul(pp, c3)
            stt(l3, 2.0, q, OP.mult, OP.add, out=l3)
            free(c1, c3, pp)
            s13 = add(l1, l3)
            l2 = stt(q, 3.0, s13, OP.mult, OP.subtract)
            free(s13, q)
            ts(l1, 0.0, OP.max, out=l1)
            ts(l2, 0.0, OP.max, out=l2)
            ts(l3, 0.0, OP.max, out=l3)
            sg1 = act(l1, AF.Sqrt)
            sg2 = act(l2, AF.Sqrt)
            sg3 = act(l3, AF.Sqrt)
            flr = ts(sg1, SIG_FLOOR, OP.mult)
            tt(sg2, flr, OP.max, out=sg2)
            tt(sg3, flr, OP.max, out=sg3)
            free(flr)

            # g-part coefficients
            f1 = recip(sg1)
            s1s2 = mul(sg1, sg2)
            s1p2 = add(sg1, sg2)
            d12 = mul(s1s2, s1p2)
            f12n = recip(d12)
            free(d12)
            num123 = add(s1p2, sg3)
            s2p3 = add(sg2, sg3)
            s3p1 = add(sg3, sg1)
            den1 = mul(s1s2, sg3)
            den2 = mul(s1p2, s2p3)
            mul(den2, s3p1, out=den2)
            mul(den1, den2, out=den1)
            f123 = recip(den1)
            mul(f123, num123, out=f123)
            free(s1s2, s1p2, num123, s2p3, s3p1, den1, den2, sg1, sg2)
            l1l2 = mul(l1, l2)
            al = mul(f123, l1l2)
            mul(f12n, l1, out=l1l2)
            add(al, l1l2, out=al)
            add(al, f1, out=al)
            free(l1l2, f1)
            bneg = add(l1, l2)
            mul(f123, bneg, out=bneg)
            add(bneg, f12n, out=bneg)
            free(f12n, l1, l2)
            ga = f123

            # M = al*I - bneg*C + ga*C2  (C2 slot becomes M slot)
            def mentry(Cij, C2ij, diag):
                mul(ga, C2ij, out=C2ij)
                x_ = mul(bneg, Cij)
                sub(C2ij, x_, out=C2ij)
                if diag:
                    add(C2ij, al, out=C2ij)
                free(x_)
                return C2ij

            M00 = mentry(C00, C2_00, True)
            M11 = mentry(C11, C2_11, True)
            M22 = mentry(C22, C2_22, True)
            gab = alloc_b()
            S.activation(gab[:], ga[:], AF.Copy)
            bnb = alloc_b()
            S.activation(bnb[:], bneg[:], AF.Copy)

            def mentry_b(Cbij, C2bij):
                mul(gab, C2bij, out=C2bij)
                x_ = mul(bnb, Cbij, out=alloc_b())
                sub(C2bij, x_, out=C2bij)
                free(x_)
                return C2bij

            M01 = mentry_b(Cb01, C2_01)
            M02 = mentry_b(Cb02, C2_02)
            M12 = mentry_b(Cb12, C2_12)
            free(al, bneg, ga, gab, bnb)

            # adjugate of (C - l3 I), Frobenius-normalized rank-1 correction
            l3b = alloc_b()
            S.activation(l3b[:], l3[:], AF.Copy)
            dd0 = sub(Cb00, l3b, out=alloc_b())
            dd1 = sub(Cb11, l3b, out=alloc_b())
            dd2 = sub(Cb22, l3b, out=alloc_b())
            free(l3b)
            s01b = sq(Cb01, out=alloc_b())
            s02b = sq(Cb02, out=alloc_b())
            s12b = sq(Cb12, out=alloc_b())
            adj00 = mul(dd1, dd2, out=alloc_b())
            sub(adj00, s12b, out=adj00)
            adj11 = mul(dd0, dd2, out=alloc_b())
            sub(adj11, s02b, out=adj11)
            adj22 = mul(dd0, dd1, out=alloc_b())
            sub(adj22, s01b, out=adj22)
            adj01 = mul(Cb01, dd2, out=alloc_b())
            tt(cpb_0212, adj01, OP.subtract, out=adj01)
            adj02 = mul(Cb02, dd1, out=alloc_b())
            tt(cpb_0112, adj02, OP.subtract, out=adj02)
            adj12 = mul(Cb12, dd0, out=alloc_b())
            tt(cpb_0102, adj12, OP.subtract, out=adj12)
            free(dd0, dd1, dd2, s01b, s02b, s12b)
            free(s01, s02, s12, cp_0212, cp_0112, cp_0102)
            free(cpb_0212, cpb_0112, cpb_0102, *Cb)
            free(C00, C01, C02, C11, C12, C22)
            fro2 = sq(adj00)
            fq = sq(adj11)
            fq2 = sq(adj22)
            osum = sq(adj01)
            fq3 = sq(adj02)
            fq4 = sq(adj12)
            with on(GP):
                add(fro2, fq, out=fro2)
                add(fro2, fq2, out=fro2)
                add(osum, fq3, out=osum)
                add(osum, fq4, out=osum)
                free(fq, fq2, fq3, fq4)
                ts(osum, 2.0, OP.mult, out=osum)
                add(fro2, osum, out=fro2)
                free(osum)
            Dc = act(fro2, AF.Sqrt)
            free(fro2)
            sd = mul(sg3, Dc)
            free(sg3, Dc, l3)
            ts(sd, 1e-35, OP.max, out=sd)
            rsd = recip(sd)
            free(sd)
            w3 = stt(sneg, -2.0, rsd, OP.mult, OP.mult)
            free(sneg, rsd)

            w3b = alloc_b()
            S.activation(w3b[:], w3[:], AF.Copy)
            free(w3)
            # convert fp32 M diag now; offdiag already bf16
            Mb00 = alloc_b()
            S.activation(Mb00[:], M00[:], AF.Copy)
            Mb11 = alloc_b()
            S.activation(Mb11[:], M11[:], AF.Copy)
            Mb22 = alloc_b()
            S.activation(Mb22[:], M22[:], AF.Copy)
            free(M00, M11, M22)
            Mb01, Mb02, Mb12 = M01, M02, M12

            def madd(Mx, adj):
                mul(w3b, adj, out=adj)
                add(Mx, adj, out=Mx)
                free(adj)

            madd(Mb00, adj00)
            madd(Mb11, adj11)
            madd(Mb22, adj22)
            madd(Mb01, adj01)
            madd(Mb02, adj02)
            madd(Mb12, adj12)
            free(w3b)

            # inner = <M, H> in bf16, with M symmetric; H row-major H[3i+j]
            Mb = [Mb00, Mb01, Mb02, Mb11, Mb12, Mb22]
            inner = mul(Mb00, H[0], out=ha())
            x = mul(Mb11, H[4], out=ha())
            add(inner, x, out=inner)
            mul(Mb22, H[8], out=x)
            add(inner, x, out=inner)
            hs = add(H[1], H[3], out=ha())
            mul(Mb01, hs, out=hs)
            add(inner, hs, out=inner)
            add(H[2], H[6], out=hs)
            mul(Mb02, hs, out=hs)
            add(inner, hs, out=inner)
            add(H[5], H[7], out=hs)
            mul(Mb12, hs, out=hs)
            add(inner, hs, out=inner)
            free(x, hs, *Mb, *H)

            # contrib = m * (6 - 2*inner)/9 ; add halves into base
            ts(inner, -2.0 / 9.0, OP.mult, 6.0 / 9.0, OP.add, out=inner)
            mul(inner, mw, out=inner)
            free(mw)
            bf1 = V.tensor_tensor(out=base[:, 0:FN], in0=base[:, 0:FN],
                                  in1=inner[:, 0:FN], op=OP.add)
            bf2 = V.tensor_tensor(out=base[:, 0:FN], in0=base[:, 0:FN],
                                  in1=inner[:, FN:FW], op=OP.add)
            _dep_on_touches(bf1, inner)
            _dep_on_touches(bf2, inner)
            free(inner)

            import sys
            if state['n'] < 400:
                print(f"ALLOC STATS: slots={state['n']} wpeak={state['peak']} apeak={state['apeak']} "
                  f"({state['peak'] * FW * 4 // 1024} KB/partition)",
                  file=sys.stderr)

            accr = pool.tile([P, 1], dt, name="accr", tag="accr", bufs=1)
            V.tensor_reduce(out=accr[:], in_=base[:, 0:FN],
                            axis=mybir.AxisListType.X, op=OP.add)
            free(base)
            nc.sync.dma_start(out=out_d.ap(), in_=accr[:])

        for _rep in range(reps):
            _once()


    nc.compile()
    return nc


def _get_nc(reps=1):
    key = f"nc{reps}-{USE_GP}-{USE_BF16}"
    if key not in _CACHE:
        _CACHE[key] = _build_nc(reps)
    return _CACHE[key]


def _shard_inputs(pred, target, pose):
    in_maps = []
    for i in range(N_CORES):
        sl = slice(i * BS, (i + 1) * BS)
        in_maps.append({
            "predT": np.ascontiguousarray(pred[sl].T.astype(np.float32)),
            "tgtT": np.ascontiguousarray(target[sl].T.astype(np.float32)),
            "poseT": np.ascontiguousarray(pose[sl, 3:6].T.astype(np.float32)),
        })
    return in_maps


def run(pred, target, pose, trace=False):
    # NOTE: trace=True requires antenv.axon_hooks (absent in this container)
    from concourse.bass_utils import run_bass_kernel_spmd
    nc = _get_nc()
    in_maps = _shard_inputs(pred, target, pose)
    res = run_bass_kernel_spmd(nc, in_maps, core_ids=list(range(N_CORES)),
                               trace=trace)
    total = 0.0
    for core_out in res.results:
        total += float(np.asarray(core_out["out"], dtype=np.float64).sum())
    loss = np.float32(total / B_TOTAL)
    return np.array(loss, dtype=np.float32), res


def kernel(pred, target, pose):
    out, _ = run(pred, target, pose)
    return out

